# revision 1
# baseline (speedup 1.0000x reference)
"""Trainium2 Bass kernel for nn_AttnAggregator (GNN message passing, 8 cores).

Strategy: data-parallel over queries. Each of the 8 NeuronCores owns 256
queries = 2560 segments. Per core, neighbors are grouped into 20 windows of
128 segments; each window's neighbor list is padded to a fixed number of
128-slot tiles (T, uniform across cores so the SPMD program is identical).

Per window:
  em      = dma_gather(ent_embeds, nbr_ids)    fp32  (GPSIMD gather)
  em16    = cast(em)                           fp16  (ACT copy)
  emT16   = xbar-DMA-transpose(em16)           fp16  [h, (tile,chunk), nbr]
Per tile [128 nbr slots]:
  z       = em16 @ W1 + c[q(n)]                (PE fp16, FWL; c added via a
                                                small one-hot matmul against
                                                an on-device fp16 c-table)
  H       = tanh(z)                            (ACT, 4-tile batches)
  score_n = sum_h H*v_s                        (DVE fused mul+reduce)
  e       = exp(score)                         (ACT, per window)
  Wm      = (iota == seg_local) * e            (DVE dual-op tensor_scalar)
  agg    += Wm.T @ em ; den += Wm.T @ ones     (PE fp32r scatter-matmul, PSUM)
Window epilogue: agg/den (masked for empty segments), gather s/r embeddings
per segment, assemble [128, 768] rows, DMA out.

The c-table c[q] = s_emb[q] @ W2 + r_emb[q] @ W3 + b is computed on-device in
a small setup phase (gather + PE transpose + matmuls).
"""

import os
import sys

import numpy as np

H = 256
SEQ_LEN = 10
NCORES = 8
WIN = 128  # segments per output window (PSUM partition dim)
SWB = 5    # windows per ss/rr gather batch


def _wrap_idx(idx_lin):
    """Wrap a linear int16 index list for dma_gather: idx i lives at
    [i % 16, i // 16], replicated across the 8 GPSIMD cores (128 rows)."""
    n = len(idx_lin)
    assert n % 16 == 0
    arr = np.asarray(idx_lin, dtype=np.int16).reshape(n // 16, 16).T  # [16, n//16]
    return np.tile(arr, (8, 1)).copy()  # [128, n//16]


def _build_core_data(c, s, r, nbr_ids, seg_ids, QPC, NW):
    """Pure-integer host-side layout work for one core's shard."""
    qlo = c * QPC
    seg_lo = qlo * SEQ_LEN
    seg_hi = (qlo + QPC) * SEQ_LEN
    lo = np.searchsorted(seg_ids, seg_lo, "left")
    hi = np.searchsorted(seg_ids, seg_hi, "left")
    segs = (seg_ids[lo:hi] - seg_lo).astype(np.int64)  # 0 .. QPC*SEQ_LEN-1
    nbrs = nbr_ids[lo:hi].astype(np.int64)

    win_bounds = [np.searchsorted(segs, w * WIN, "left") for w in range(NW + 1)]
    cnts = [win_bounds[w + 1] - win_bounds[w] for w in range(NW)]
    tiles = [max(1, -(-cnt // 128)) for cnt in cnts]
    return segs, nbrs, win_bounds, cnts, tiles


def kernel(s, r, nbr_ids, seg_ids, ent_embeds, rel_embeds, W_attn, b_attn, v_s):
    sys.path.insert(0, "/opt/trn_rl_repo")
    import concourse.bass as bass  # noqa: F401
    import concourse.tile as tile
    from concourse import bacc, mybir
    from concourse.bass_utils import run_bass_kernel_spmd
    from contextlib import ExitStack

    f32 = mybir.dt.float32
    f32r = mybir.dt.float32r
    f16 = mybir.dt.float16
    i16 = mybir.dt.int16
    AF = mybir.ActivationFunctionType
    OP = mybir.AluOpType

    s = np.asarray(s)
    r = np.asarray(r)
    nbr_ids = np.asarray(nbr_ids)
    seg_ids = np.asarray(seg_ids)
    ent_embeds = np.ascontiguousarray(np.asarray(ent_embeds, dtype=np.float32))
    rel_embeds = np.ascontiguousarray(np.asarray(rel_embeds, dtype=np.float32))
    W_attn = np.asarray(W_attn, dtype=np.float32)
    b_attn = np.asarray(b_attn, dtype=np.float32)
    v_s = np.asarray(v_s, dtype=np.float32).reshape(-1)

    B = s.shape[0]
    NUM_SEG = B * SEQ_LEN
    QPC = B // NCORES              # queries per core
    SPC = QPC * SEQ_LEN            # segments per core
    NW = SPC // WIN                # windows per core

    # ---------------- host-side integer layout ----------------
    per_core = [
        _build_core_data(c, s, r, nbr_ids, seg_ids, QPC, NW) for c in range(NCORES)
    ]
    T = max(max(t) for (_, _, _, _, t) in per_core)  # tiles per window (uniform)
    SLOTS = NW * T * 128

    counts_all = np.bincount(np.asarray(seg_ids, dtype=np.int64), minlength=NUM_SEG)

    # Per-WINDOW query base for the c-add one-hot matmul (uniform across
    # cores: computed from w alone). Window w covers local queries
    # [floor(w*128/10), floor(((w+1)*128-1)/10)] — span <= 14 = KQW.
    QW = []  # (qbase, kq) per window
    KQW = WIN // SEQ_LEN + 2  # 14: max queries touched by one window
    for w in range(NW):
        qb = (w * WIN) // SEQ_LEN
        kq = min(KQW, QPC - qb)
        QW.append((qb, kq))

    in_maps = []
    for c in range(NCORES):
        segs, nbrs, wb, cnts, _tiles = per_core[c]
        em_idx = np.zeros(SLOTS, dtype=np.int64)
        segloc = np.full((NW * T, 128), 255.0, dtype=np.float32)  # [tile, part]
        qloc = np.full(SLOTS, -1, dtype=np.int64)
        for w in range(NW):
            cnt = cnts[w]
            base = w * T * 128
            em_idx[base : base + cnt] = nbrs[wb[w] : wb[w + 1]]
            sl = segs[wb[w] : wb[w + 1]] - w * WIN
            tl = np.full(T * 128, 255.0, dtype=np.float32)
            tl[:cnt] = sl.astype(np.float32)
            segloc[w * T : (w + 1) * T, :] = tl.reshape(T, 128)
            qloc[base : base + cnt] = (segs[wb[w] : wb[w + 1]] // SEQ_LEN)

        qoh = np.zeros((KQW, SLOTS), dtype=np.float16)
        for w in range(NW):
            qb = QW[w][0]
            sl = slice(w * T * 128, (w + 1) * T * 128)
            ql = qloc[sl]
            rel_q = np.where(ql >= 0, ql - qb, -1)
            assert rel_q.max() < KQW
            for k in range(KQW):
                qoh[k, sl] = (rel_q == k).astype(np.float16)

        # per-segment arrays
        seg_global0 = c * SPC
        segq = (np.arange(SPC) // SEQ_LEN) + c * QPC  # global query per local seg
        sw_idx = s[segq].astype(np.int64)  # ent row per local seg
        rw_idx = r[segq].astype(np.int64)
        cnts_core = counts_all[seg_global0 : seg_global0 + SPC]
        maskw = (cnts_core > 0).astype(np.float32).reshape(NW, 128).T  # [128, NW]
        invw = 1.0 - maskw

        sq = s[c * QPC : (c + 1) * QPC].astype(np.int64)  # [QPC]
        rq = r[c * QPC : (c + 1) * QPC].astype(np.int64)

        im = {
            "ent": ent_embeds,
            "rel": rel_embeds,
            "wq1": W_attn[0:256].reshape(2, 128, 256).transpose(1, 0, 2)
                   .astype(np.float16).copy(),
            "wq2": W_attn[256:512].reshape(2, 128, 256).transpose(1, 0, 2).copy(),
            "wq3": W_attn[512:768].reshape(2, 128, 256).transpose(1, 0, 2).copy(),
            "b_row": b_attn.reshape(1, 256).copy(),
            "vbc": np.tile(v_s, (128, 1)),
            "ones2": np.ones((128, 2), dtype=np.float32),
            "ones_row": np.ones((1, 128), dtype=np.float32),
            "ident": np.eye(128, dtype=np.float32),
            "iota": np.tile(np.arange(128, dtype=np.float32), (128, 1)),
            "em_idx": _wrap_idx(em_idx),
            "sq_idx": _wrap_idx(sq),
            "rq_idx": _wrap_idx(rq),
            "sw_idx": _wrap_idx(sw_idx),
            "rw_idx": _wrap_idx(rw_idx),
            "segl": np.ascontiguousarray(segloc.T),  # [128, NW*T]
            "qoh": qoh,
            "maskw": np.ascontiguousarray(maskw),
            "invw": np.ascontiguousarray(invw),
        }
        in_maps.append(im)

    # ---------------- build the SPMD program ----------------
    print("[kernel] host prep done", flush=True)
    nc = bacc.Bacc("TRN2", target_bir_lowering=False, debug=False,
                   num_devices=NCORES, num_swdge_queues=4)

    def din(name, shape, dt):
        return nc.dram_tensor(name, shape, dt, kind="ExternalInput").ap()

    ent_ap = din("ent", [ent_embeds.shape[0], 256], f32)
    rel_ap = din("rel", [rel_embeds.shape[0], 256], f32)
    wq1_ap = din("wq1", [128, 2, 256], f16)
    wq2_ap = din("wq2", [128, 2, 256], f32)
    wq3_ap = din("wq3", [128, 2, 256], f32)
    brow_ap = din("b_row", [1, 256], f32)
    vbc_ap = din("vbc", [128, 256], f32)
    ones2_ap = din("ones2", [128, 2], f32)
    onesr_ap = din("ones_row", [1, 128], f32)
    ident_ap = din("ident", [128, 128], f32)
    iota_ap = din("iota", [128, 128], f32)
    emidx_ap = din("em_idx", [128, SLOTS // 16], i16)
    sqidx_ap = din("sq_idx", [128, QPC // 16], i16)
    rqidx_ap = din("rq_idx", [128, QPC // 16], i16)
    swidx_ap = din("sw_idx", [128, SPC // 16], i16)
    rwidx_ap = din("rw_idx", [128, SPC // 16], i16)
    segl_ap = din("segl", [128, NW * T], f32)
    qoh_ap = din("qoh", [KQW, SLOTS], f16)
    maskw_ap = din("maskw", [128, NW], f32)
    invw_ap = din("invw", [128, NW], f32)
    out_ap = nc.dram_tensor("out", [SPC, 768], f32, kind="ExternalOutput").ap()

    import itertools as _it
    _swq_counter = _it.count()

    def _swq():
        return next(_swq_counter) % 4

    _patch_swdge_lane_assignment()

    import time as _time
    _t0 = _time.time()
    with tile.TileContext(nc) as tc, ExitStack() as ctx:
        cons = ctx.enter_context(tc.tile_pool(name="cons", bufs=1))
        emp = ctx.enter_context(tc.tile_pool(name="emp", bufs=3))
        emq = ctx.enter_context(tc.tile_pool(name="emq", bufs=2))
        work = ctx.enter_context(tc.tile_pool(name="work", bufs=3))
        outp = ctx.enter_context(tc.tile_pool(name="outp", bufs=2))
        ps_z = ctx.enter_context(tc.tile_pool(name="ps_z", bufs=2, space="PSUM"))
        ps_a = ctx.enter_context(tc.tile_pool(name="ps_a", bufs=2, space="PSUM"))
        ps_d = ctx.enter_context(tc.tile_pool(name="ps_d", bufs=2, space="PSUM"))

        # resident constants
        def cload(tag, shape, dt, ap, cast=False):
            t = cons.tile(shape, dt, tag=tag)
            nc.sync.dma_start(t[:], (ap.bitcast(dt) if cast else ap)[:])
            return t

        wq1 = cload("wq1", [128, 2, 256], f16, wq1_ap)
        wq2 = cload("wq2", [128, 2, 256], f32r, wq2_ap, cast=True)
        wq3 = cload("wq3", [128, 2, 256], f32r, wq3_ap, cast=True)
        brow = cload("brow", [1, 256], f32r, brow_ap, cast=True)
        vbc = cload("vbc", [128, 256], f32, vbc_ap)
        onesr = cload("onesr", [1, 128], f32r, onesr_ap, cast=True)
        ident = cload("ident", [128, 128], f32r, ident_ap, cast=True)
        iota = cload("iota", [128, 128], f32, iota_ap)
        segl = cload("segl", [128, NW * T], f32, segl_ap)
        ones2 = cload("ones2", [128, 2], f32r, ones2_ap, cast=True)
        emidx = cload("emidx", [128, SLOTS // 16], i16, emidx_ap)
        sqidx = cload("sqidx", [128, QPC // 16], i16, sqidx_ap)
        rqidx = cload("rqidx", [128, QPC // 16], i16, rqidx_ap)
        swidx = cload("swidx", [128, SPC // 16], i16, swidx_ap)
        rwidx = cload("rwidx", [128, SPC // 16], i16, rwidx_ap)
        maskw = cload("maskw", [128, NW], f32, maskw_ap)
        invw = cload("invw", [128, NW], f32, invw_ap)

        # ---- setup: c-table c[q] = s_emb[q] @ W2 + r_emb[q] @ W3 + b ----
        s_emb = cons.tile([128, QPC // 128, 256], f32r)
        nc.gpsimd.dma_gather(s_emb[:], ent_ap.bitcast(f32r)[:], sqidx[:],
                             num_idxs=QPC, num_idxs_reg=QPC, elem_size=256,
                             single_packet=False, queue_num=_swq())
        r_emb = cons.tile([128, QPC // 128, 256], f32r)
        nc.gpsimd.dma_gather(r_emb[:], rel_ap.bitcast(f32r)[:], rqidx[:],
                             num_idxs=QPC, num_idxs_reg=QPC, elem_size=256,
                             single_packet=False, queue_num=_swq())

        sT = cons.tile([128, 2, 256], f32r)   # [h, hc, q]
        rT = cons.tile([128, 2, 256], f32r)
        for src, dstT in ((s_emb, sT), (r_emb, rT)):
            tp = ps_a.tile([128, 2, 256], f32r, tag="agg")
            for qc in range(2):
                for hc in range(2):
                    nc.tensor.transpose(tp[:, hc, qc * 128:(qc + 1) * 128],
                                        src[:, qc, hc * 128:(hc + 1) * 128],
                                        ident[:])
            nc.scalar.copy(dstT[:], tp[:])

        # c rows for each window's query span, at partition base 0 (fp16)
        c_win = cons.tile([32, NW, 256], f16)
        for w in range(NW):
            qb, kq = QW[w]
            cp = ps_z.tile([128, 4, 256], f32, tag="z")
            for hc in range(2):
                nc.tensor.matmul(cp[0:kq, 0, :], sT[:, hc, qb:qb + kq],
                                 wq2[:, hc, :], start=(hc == 0), stop=False)
            for hc in range(2):
                nc.tensor.matmul(cp[0:kq, 0, :], rT[:, hc, qb:qb + kq],
                                 wq3[:, hc, :], start=False, stop=False)
            nc.tensor.matmul(cp[0:kq, 0, :], onesr[:, 0:kq], brow[:],
                             start=False, stop=True)
            nc.scalar.copy(c_win[0:kq, w, :], cp[0:kq, 0, :])

        # ---- main loop over windows ----
        NW_RUN = int(os.environ.get("KERNEL_NWIN", str(NW)))
        ssb = rrb = None
        for w in range(NW_RUN):
            em_w = emp.tile([128, T, 256], f32r, tag="em")
            em16 = emq.tile([128, T, 256], f16, tag="em16")
            emT16 = emq.tile([128, 2 * T, 128], f16, tag="emT16")
            NQ = 3
            tparts = [(T * p // NQ, T * (p + 1) // NQ) for p in range(NQ)]
            for p, (tlo, thi) in enumerate(tparts):
                nt = thi - tlo
                nc.gpsimd.dma_gather(
                    em_w[:, tlo:thi, :], ent_ap.bitcast(f32r)[:],
                    emidx[:, (w * T + tlo) * 8:(w * T + thi) * 8],
                    num_idxs=nt * 128, num_idxs_reg=nt * 128, elem_size=256,
                    single_packet=False, queue_num=_swq())
                nc.scalar.copy(em16[:, tlo:thi, :],
                               em_w.bitcast(f32)[:, tlo:thi, :])
                nc.sync.dma_start(emT16[:, 2 * tlo:2 * thi, :],
                                  em16[:, tlo:thi, :], transpose=True)
            qoh_w = emq.tile([KQW, T * 128], f16, tag="qoh")
            nc.sync.dma_start(qoh_w[:],
                              qoh_ap[:, w * T * 128:(w + 1) * T * 128])

            if w % SWB == 0:
                nsw = min(SWB, NW_RUN - w)
                ssb = outp.tile([128, SWB, 256], f32, tag="ssb")
                nc.gpsimd.dma_gather(
                    ssb[:, 0:nsw, :], ent_ap[:],
                    swidx[:, w * 8:(w + nsw) * 8],
                    num_idxs=nsw * 128, num_idxs_reg=nsw * 128, elem_size=256,
                    single_packet=False, queue_num=_swq())
                rrb = outp.tile([128, SWB, 256], f32, tag="rrb")
                nc.gpsimd.dma_gather(
                    rrb[:, 0:nsw, :], rel_ap[:],
                    rwidx[:, w * 8:(w + nsw) * 8],
                    num_idxs=nsw * 128, num_idxs_reg=nsw * 128, elem_size=256,
                    single_packet=False, queue_num=_swq())

            scores = work.tile([128, T], f32, tag="scores")
            ebuf = work.tile([128, T], f32, tag="ebuf")
            agg_ps = ps_a.tile([128, 256], f32, tag="agg")
            den_ps = ps_d.tile([128, 2], f32, tag="den")

            qb, kq = QW[w]
            ngrp = (T + 3) // 4
            for g in range(ngrp):
                t0 = g * 4
                nt = min(4, T - t0)
                zp = ps_z.tile([128, 4, 256], f32, tag="z")
                for t in range(t0, t0 + nt):
                    tg = t - t0
                    zps = zp[:, tg, :]
                    for hc in range(2):
                        nc.tensor.matmul(zps, emT16[:, 2 * t + hc, :],
                                         wq1[:, hc, :], start=(hc == 0),
                                         stop=False)
                    nc.tensor.matmul(zps,
                                     qoh_w[0:kq, t * 128:(t + 1) * 128],
                                     c_win[0:kq, w, :],
                                     start=False, stop=True)
                Hsb = work.tile([128, 4, 256], f32, tag="H")
                nc.scalar.activation(Hsb[:, 0:nt, :], zp[:, 0:nt, :], AF.Tanh)
                for t in range(t0, t0 + nt):
                    tg = t - t0
                    hv = work.tile([128, 256], f32, tag="hv")
                    nc.vector.scalar_tensor_tensor(
                        hv[:], Hsb[:, tg, :], 1.0, vbc[:], OP.mult, OP.mult,
                        accum_out=scores[:, t:t + 1])

            nc.scalar.activation(ebuf[:], scores[:], AF.Exp)

            for t in range(T):
                wm = work.tile([128, 128], f32r, tag="wm")
                nc.vector.tensor_scalar(wm[:], iota[:],
                                        segl[:, w * T + t:w * T + t + 1],
                                        ebuf[:, t:t + 1],
                                        op0=OP.is_equal, op1=OP.mult)
                nc.tensor.matmul(agg_ps[:], wm[:], em_w[:, t, :],
                                 start=(t == 0), stop=(t == T - 1))
                nc.tensor.matmul(den_ps[:], wm[:], ones2[:],
                                 start=(t == 0), stop=(t == T - 1))

            # window epilogue
            dtmp = work.tile([128, 1], f32, tag="dtmp")
            nc.vector.tensor_add(dtmp[:], den_ps[:, 0:1], invw[:, w:w + 1])
            dinv = work.tile([128, 1], f32, tag="dinv")
            nc.vector.reciprocal(dinv[:], dtmp[:])

            out_sb = outp.tile([128, 768], f32, tag="out")
            nc.scalar.activation(out_sb[:, 0:256], agg_ps[:, 0:256], AF.Copy,
                                 scale=dinv[:])
            nc.vector.tensor_scalar_mul(out_sb[:, 256:512], ssb[:, w % SWB, :],
                                        maskw[:, w:w + 1])
            nc.vector.tensor_scalar_mul(out_sb[:, 512:768], rrb[:, w % SWB, :],
                                        maskw[:, w:w + 1])
            nc.sync.dma_start(out_ap[w * 128:(w + 1) * 128, :], out_sb[:])

    print(f"[kernel] program built+scheduled in {_time.time()-_t0:.1f}s",
          flush=True)
    nc.compile()
    print("[kernel] bacc.compile done; launching", flush=True)

    if os.environ.get("KERNEL_SIM"):
        from concourse.bass_interp import CoreSim
        sim = CoreSim(nc, trace=False)
        for k, v in in_maps[0].items():
            sim.tensor(k)[:] = v
        sim.simulate(check_with_hw=False)
        print("[kernel] CoreSim passed", flush=True)
        import types
        res = types.SimpleNamespace(
            results=[{"out": np.array(sim.tensor("out"))} for _ in range(NCORES)],
            exec_time_ns=None)
        out = np.concatenate([res.results[c]["out"] for c in range(NCORES)], axis=0)
        return out.reshape(B, SEQ_LEN, 3 * H)

    trace = bool(int(os.environ.get("KERNEL_TRACE", "0")))
    if trace:
        _install_prof_hook()
    res = run_bass_kernel_spmd(nc, in_maps, list(range(NCORES)), trace=trace)
    if trace and res.exec_time_ns is not None:
        print(f"HW exec time: {res.exec_time_ns} ns")

    out = np.concatenate([res.results[c]["out"] for c in range(NCORES)], axis=0)
    return out.reshape(B, SEQ_LEN, 3 * H)


def _patch_swdge_lane_assignment():
    """Make Tile's DMASW completion-sem lane choice queue-aware so SWDGE
    multi-queue DMAs don't share a semaphore lane across queues (each sem is
    locked to the queue that first increments it). Lanes 2q and 2q+1 serve
    queue q."""
    import concourse.tile_sem_assignment as tsa
    import concourse.mybir as mybir

    cls = tsa.TileClockTick
    if getattr(cls, "_swq_patched", False):
        return
    orig = cls._assign_tick

    def _assign_tick(self, inst):
        if (
            isinstance(inst, tsa.DMAInst)
            and inst.engine == mybir.EngineType.Pool
        ):
            q = getattr(inst, "queue_num", 0) or 0
            if not hasattr(self, "_swq_rot"):
                self._swq_rot = {}
            rot = self._swq_rot.get(q, 0)
            self._swq_rot[q] = rot ^ 1
            lane = (2 * q + rot) % self.swdge_sem_count
            save = self.next_sw_dma_idx
            self.next_sw_dma_idx = lane
            try:
                return orig(self, inst)
            finally:
                self.next_sw_dma_idx = save
        return orig(self, inst)

    cls._assign_tick = _assign_tick
    cls._swq_patched = True


def _install_prof_hook():
    """Shim antenv.axon_hooks so trace=True can NTFF-profile under axon."""
    import contextlib
    import ctypes
    import types

    import antenv

    if "antenv.axon_hooks" in sys.modules:
        return
    so = "/opt/axon/libaxon_pjrt.so"
    lib = ctypes.CDLL(so)
    if not hasattr(lib, "axon_start_nrt_profile"):
        return
    lib.axon_start_nrt_profile.argtypes = [ctypes.POINTER(ctypes.c_int64),
                                           ctypes.c_size_t]
    lib.axon_start_nrt_profile.restype = ctypes.c_int64
    lib.axon_stop_nrt_profile.argtypes = [ctypes.c_char_p]
    lib.axon_stop_nrt_profile.restype = ctypes.c_int64

    @contextlib.contextmanager
    def _hook(output_dir, device_ids):
        import jax

        jax.devices()
        if device_ids:
            ids = (ctypes.c_int64 * len(device_ids))(*device_ids)
            rc = lib.axon_start_nrt_profile(ids, len(device_ids))
        else:
            rc = lib.axon_start_nrt_profile(None, 0)
        if rc != 0:
            raise RuntimeError(f"axon_start_nrt_profile rc={rc}")
        try:
            yield
        finally:
            n = lib.axon_stop_nrt_profile(str(output_dir).encode())
            print(f"profile: {n} file(s) written to {output_dir}",
                  file=sys.stderr)

    mod = types.ModuleType("antenv.axon_hooks")
    mod.get_axon_ntff_profile_hook = lambda: _hook
    mod.set_axon_ntff_profile_hook = lambda h: None
    sys.modules["antenv.axon_hooks"] = mod
    antenv.axon_hooks = mod

